# revision 14
# baseline (speedup 1.0000x reference)
"""Trainium2 Bass kernel for SNN Linear(2048->1024) + snntorch Leaky (LIF)
layer over T=100 timesteps.

  cur = einsum('tbi,oi->tbo', x, W)
  mem_t = beta*mem_{t-1} + cur_t - heaviside(mem_{t-1} - 1)
  spk_t = heaviside(mem_t - 1)
  returns (spk_rec, mem_rec), both [T, B, NO] float32

Strategy (hardcoded for x:[100,128,2048] f32, W:[1024,2048] f32, 8 cores):
  - Data-parallel over batch: core c handles b in [16c, 16c+16). W replicated.
  - Host pre-transposes x to [core, NI, T, 16] and W to [NI, NO] so both
    matmul operands arrive with the contraction dim on partitions.
  - Matmul runs on TensorE with the weight tile stationary; out tile is
    [o_chunk(128), (t,b)] in PSUM, N=t_block*16 per tile (>=256 keeps
    float32r at full rate).
  - MODE "f32r": single-pass float32r matmuls (TRN2 rounds inputs to 11
    mantissa bits, RTN). MODE "bf16x3": hi/lo bf16 split of both operands,
    3 accumulating passes (drops only the lo*lo term) for ~17-bit mantissa.
  - ScalarE copies PSUM->SBUF into a [128, t, oc, b] block so each timestep
    slice is a contiguous [128,128] tile; VectorE runs the serial LIF
    recurrence as 3 elementwise ops per timestep (same fp32 op order as the
    reference); results stream out per block on the ACT DMA ring.
  - Host re-transposes outputs to [T, B, NO].
"""

import numpy as np
import ml_dtypes

import concourse.bass as bass
import concourse.mybir as mybir
import concourse.tile as tile
from concourse import bacc, bass2jax

# Problem constants (hardcoded per spec)
T, B, NI, NO = 100, 128, 2048, 1024
NCORES = 8
BS = B // NCORES          # 16 batch rows per core
KC = NI // 128            # 16 contraction chunks
OC = NO // 128            # 8 output chunks
BLOCKS = [20, 20, 20, 20, 20]   # timesteps per pipeline block (sum=100)
BETA = 0.9
THRESHOLD = 1.0

MODE = "f32rx3"           # "f32r" (1-pass) or "bf16x3" (hi/lo split, 3-pass)

_cache = {}


def _build_nc(mode, reps=1):
    """reps>1 wraps the whole pipeline in a runtime loop (timing only)."""
    assert sum(BLOCKS) == T and all(b * BS >= 256 for b in BLOCKS)
    f32 = mybir.dt.float32
    nc = bacc.Bacc(None, target_bir_lowering=False)

    if mode in ("f32r", "f32"):
        mmdt = mybir.dt.float32r if mode == "f32r" else mybir.dt.float32
        d_w = [nc.dram_tensor("wT", (NI, NO), mmdt, kind="ExternalInput")]
        d_x = [nc.dram_tensor("xT", (NI, T, BS), mmdt, kind="ExternalInput")]
    else:
        mmdt = mybir.dt.bfloat16
        d_w = [nc.dram_tensor("wTh", (NI, NO), mmdt, kind="ExternalInput"),
               nc.dram_tensor("wTl", (NI, NO), mmdt, kind="ExternalInput")]
        d_x = [nc.dram_tensor("xTh", (NI, T, BS), mmdt, kind="ExternalInput"),
               nc.dram_tensor("xTl", (NI, T, BS), mmdt, kind="ExternalInput")]
    # output layout [p, t, oc, b]: contiguous per-partition streams from the
    # [128, t, (oc b)] SBUF blocks
    d_spk = nc.dram_tensor("spk", (128, T, OC, BS), f32, kind="ExternalOutput")
    d_mem = nc.dram_tensor("mem", (128, T, OC, BS), f32, kind="ExternalOutput")

    # matmul passes: list of (w_tensor_idx, x_tensor_idx)
    passes = [(0, 0)] if mode in ("f32r", "f32") else [(0, 0), (0, 1), (1, 0)]
    tmax = max(BLOCKS)

    with tile.TileContext(nc) as tc:
        with (
            tc.tile_pool(name="w", bufs=1) as wpool,
            tc.tile_pool(name="x", bufs=2) as xpool,
            tc.tile_pool(name="cur", bufs=2) as cpool,
            tc.tile_pool(name="ospk", bufs=2) as spool,
            tc.tile_pool(name="omem", bufs=2) as mpool,
            tc.tile_pool(name="state", bufs=1) as tpool,
            tc.tile_pool(name="psum", bufs=8, space="PSUM") as ppool,
        ):
            w_sb = []
            for wi in range(len(d_w)):
                wt = wpool.tile([128, KC, NO], mmdt, tag=f"w{wi}", name=f"w{wi}")
                w_sb.append(wt)

            def load_w():
                for wi, dw in enumerate(d_w):
                    dvw = dw.ap().rearrange("(c p) m -> p c m", p=128)
                    for c in range(KC):
                        nc.sync.dma_start(out=w_sb[wi][:, c, :], in_=dvw[:, c, :])

            zeros = tpool.tile([128, OC * BS], f32, tag="zeros")
            nc.vector.memset(zeros[:], 0.0)
            m1 = tpool.tile([128, OC * BS], f32, tag="m1")

            def pipeline():
                pm = zeros[:]   # mem_{t-1} slice (starts as zeros)
                ps = zeros[:]   # spk_{t-1} slice
                t0 = 0
                for tblk in BLOCKS:
                    nb = tblk * BS
                    x_sb = []
                    for xi, dx in enumerate(d_x):
                        xt = xpool.tile([128, KC, tmax, BS], mmdt, tag=f"x{xi}")
                        x_sb.append(xt)
                        nc.sync.dma_start(
                            out=xt[:, :, :tblk, :],
                            in_=dx.ap().rearrange("(c p) t b -> p c t b", p=128)[:, :, t0:t0 + tblk, :],
                        )
                    # cur block: oc-major so ACT copies are contiguous
                    cur_sb = cpool.tile([128, OC, tmax, BS], f32, tag="cur")
                    for oc in range(OC):
                        acc = ppool.tile([128, tmax * BS], f32, tag="acc")
                        n_mm = len(passes) * KC
                        i_mm = 0
                        for (wi, xi) in passes:
                            for c in range(KC):
                                nc.tensor.matmul(
                                    acc[:, :nb],
                                    w_sb[wi][:, c, oc * 128:(oc + 1) * 128],
                                    x_sb[xi][:, c, :tblk, :].rearrange("p t b -> p (t b)"),
                                    start=(i_mm == 0), stop=(i_mm == n_mm - 1),
                                )
                                i_mm += 1
                        nc.scalar.copy(
                            out=cur_sb[:, oc, :tblk, :].rearrange("p t b -> p (t b)"),
                            in_=acc[:, :nb],
                        )

                    spk_buf = spool.tile([128, tmax, OC * BS], f32, tag="spk")
                    mem_buf = mpool.tile([128, tmax, OC * BS], f32, tag="mem")
                    for tt in range(tblk):
                        cur_t = cur_sb[:, :, tt, :]   # [128, OC, BS] 3D AP
                        mem_t = mem_buf[:, tt]
                        spk_t = spk_buf[:, tt]
                        # m1 = beta * mem_{t-1} + cur_t
                        nc.vector.scalar_tensor_tensor(
                            out=m1[:], in0=pm, scalar=BETA, in1=cur_t,
                            op0=mybir.AluOpType.mult, op1=mybir.AluOpType.add,
                        )
                        # mem_t = m1 - spk_{t-1}   (reset-by-subtract, thr=1)
                        nc.vector.tensor_tensor(
                            out=mem_t, in0=m1[:], in1=ps, op=mybir.AluOpType.subtract
                        )
                        # spk_t = mem_t > 1.0
                        nc.vector.tensor_scalar(
                            out=spk_t, in0=mem_t, scalar1=THRESHOLD, scalar2=None,
                            op0=mybir.AluOpType.is_gt,
                        )
                        pm = mem_t
                        ps = spk_t

                    # stream block out on the ACT HWDGE ring (separate from
                    # the input ring on SP); fully contiguous per partition
                    nc.scalar.dma_start(
                        out=d_spk.ap()[:, t0:t0 + tblk].rearrange("p t o b -> p t (o b)"),
                        in_=spk_buf[:, :tblk],
                    )
                    nc.scalar.dma_start(
                        out=d_mem.ap()[:, t0:t0 + tblk].rearrange("p t o b -> p t (o b)"),
                        in_=mem_buf[:, :tblk],
                    )
                    t0 += tblk

            if reps == 1:
                load_w()
                pipeline()
            else:
                with tc.For_i(0, reps, 1):
                    load_w()
                    pipeline()

    nc.compile()
    return nc


def _build_nc_f32rx3(reps=1):
    """f32r hi/lo split, 3 accumulating passes, o-halved so both W forms fit
    in SBUF.

    TRN2 rounds float32r values to 11 mantissa bits (RTN) at every f32r
    memory write/read, so DMA-ing the raw fp32 bits into an f32r tile yields
    the hi part for free; the host ships the lo residual (x - RTN11(x)) as a
    second f32r stream.  cur = W_hi.x_hi + W_hi.x_lo + W_lo.x_hi drops only
    the ~2^-24 lo.lo term: fp32-class precision at 3 cyc/row instead of
    native fp32's 4, and full moving-rate at N>=256.
    """
    f32 = mybir.dt.float32
    f32r = mybir.dt.float32r
    nc = bacc.Bacc(None, target_bir_lowering=False)

    d_w = nc.dram_tensor("wT", (NI, NO), f32r, kind="ExternalInput")
    d_wl = nc.dram_tensor("wTl", (NI, NO), f32r, kind="ExternalInput")
    d_x = nc.dram_tensor("xT", (NI, T, BS), f32r, kind="ExternalInput")
    d_xl = nc.dram_tensor("xTl", (NI, T, BS), f32r, kind="ExternalInput")
    d_spk = nc.dram_tensor("spk", (128, T, OC, BS), f32, kind="ExternalOutput")
    d_mem = nc.dram_tensor("mem", (128, T, OC, BS), f32, kind="ExternalOutput")

    OH = OC // 2           # 4 o-chunks per half
    NOH = NO // 2          # 512
    tmax = max(BLOCKS)

    with tile.TileContext(nc) as tc:
        with (
            tc.tile_pool(name="whi", bufs=2) as wpool_hi,
            tc.tile_pool(name="wlo", bufs=1) as wpool_lo,
            tc.tile_pool(name="x", bufs=2) as xpool,
            tc.tile_pool(name="cur", bufs=1) as cpool,
            tc.tile_pool(name="ospk", bufs=2) as spool,
            tc.tile_pool(name="omem", bufs=2) as mpool,
            tc.tile_pool(name="state", bufs=1) as tpool,
            tc.tile_pool(name="psum", bufs=8, space="PSUM") as ppool,
        ):
            zeros = tpool.tile([128, OH * BS], f32, tag="zeros")
            nc.vector.memset(zeros[:], 0.0)
            m1 = tpool.tile([128, OH * BS], f32, tag="m1")

            def sweep(half):
                o0 = half * NOH
                # whi double-buffered: sweep 1's hi-half prefetches during
                # sweep 0. wlo streams at the boundary; kc-outer consumption
                # tolerates its arrival rate.
                w_hi = wpool_hi.tile([128, KC, NOH], f32r, tag="whi", name="whi")
                w_lo = wpool_lo.tile([128, KC, NOH], f32r, tag="wlo", name="wlo")
                dvw = d_w.ap().rearrange("(c p) m -> p c m", p=128)
                dvwl = d_wl.ap().rearrange("(c p) m -> p c m", p=128)
                for c in range(KC):
                    nc.sync.dma_start(out=w_hi[:, c, :], in_=dvw[:, c, o0:o0 + NOH])
                    nc.sync.dma_start(out=w_lo[:, c, :], in_=dvwl[:, c, o0:o0 + NOH])

                pm = zeros[:]
                ps = zeros[:]
                t0 = 0
                for tblk in BLOCKS:
                    nb = tblk * BS
                    x_hi = xpool.tile([128, KC, tmax, BS], f32r, tag="xhi")
                    x_lo = xpool.tile([128, KC, tmax, BS], f32r, tag="xlo")
                    # split per 4-kc group so first matmuls start after ~1/4
                    # of the block's bytes instead of all of them
                    for cg in range(0, KC, 4):
                        for (xt, dx) in ((x_hi, d_x), (x_lo, d_xl)):
                            nc.sync.dma_start(
                                out=xt[:, cg:cg + 4, :tblk, :],
                                in_=dx.ap().rearrange("(c p) t b -> p c t b", p=128)[:, cg:cg + 4, t0:t0 + tblk, :],
                            )

                    cur_sb = cpool.tile([128, OH, tmax, BS], f32, tag="cur")
                    accs = [ppool.tile([128, tmax * BS], f32, tag="acc",
                                       name=f"acc{oc}") for oc in range(OH)]
                    # kc-outer so W/x chunks are consumed at their DMA
                    # arrival rate (removes sweep-start gating)
                    for c in range(KC):
                        for oc in range(OH):
                            for pi, (wt, xt) in enumerate(
                                ((w_hi, x_hi), (w_hi, x_lo), (w_lo, x_hi))
                            ):
                                nc.tensor.matmul(
                                    accs[oc][:, :nb],
                                    wt[:, c, oc * 128:(oc + 1) * 128],
                                    xt[:, c, :tblk, :].rearrange("p t b -> p (t b)"),
                                    start=(c == 0 and pi == 0),
                                    stop=(c == KC - 1 and pi == 2),
                                )
                    for oc in range(OH):
                        nc.scalar.copy(
                            out=cur_sb[:, oc, :tblk, :].rearrange("p t b -> p (t b)"),
                            in_=accs[oc][:, :nb],
                        )

                    spk_buf = spool.tile([128, tmax, OH * BS], f32, tag="spk")
                    mem_buf = mpool.tile([128, tmax, OH * BS], f32, tag="mem")
                    for tt in range(tblk):
                        cur_t = cur_sb[:, :, tt, :]   # [128, OH, BS] 3D AP
                        mem_t = mem_buf[:, tt]
                        spk_t = spk_buf[:, tt]
                        nc.vector.scalar_tensor_tensor(
                            out=m1[:], in0=pm, scalar=BETA, in1=cur_t,
                            op0=mybir.AluOpType.mult, op1=mybir.AluOpType.add,
                        )
                        nc.vector.tensor_tensor(
                            out=mem_t, in0=m1[:], in1=ps, op=mybir.AluOpType.subtract
                        )
                        nc.vector.tensor_scalar(
                            out=spk_t, in0=mem_t, scalar1=THRESHOLD, scalar2=None,
                            op0=mybir.AluOpType.is_gt,
                        )
                        pm = mem_t
                        ps = spk_t

                    th = tblk // 2
                    for (lo, hi) in ((0, th), (th, tblk)):
                        nc.scalar.dma_start(
                            out=d_spk.ap()[:, t0 + lo:t0 + hi, 4 * half:4 * half + OH, :]
                                .rearrange("p t o b -> p t (o b)"),
                            in_=spk_buf[:, lo:hi],
                        )
                        nc.scalar.dma_start(
                            out=d_mem.ap()[:, t0 + lo:t0 + hi, 4 * half:4 * half + OH, :]
                                .rearrange("p t o b -> p t (o b)"),
                            in_=mem_buf[:, lo:hi],
                        )
                    t0 += tblk

            def whole():
                sweep(0)
                sweep(1)

            if reps == 1:
                whole()
            else:
                with tc.For_i(0, reps, 1):
                    whole()

    nc.compile()
    return nc


def _get_nc(mode):
    if mode not in _cache:
        _cache[mode] = (_build_nc_f32rx3() if mode == "f32rx3" else _build_nc(mode))
    return _cache[mode]


def _get_runner(mode):
    """Jitted SPMD callable for the mode's Bass program, cached across calls.

    Mirrors concourse.bass_utils.run_bass_kernel_spmd's axon path but keeps
    the jax.jit object alive so repeated kernel() calls skip retracing.
    """
    key = ("runner", mode)
    if key in _cache:
        return _cache[key]
    import jax
    from jax.experimental.shard_map import shard_map
    from jax.sharding import Mesh, PartitionSpec

    nc = _get_nc(mode)
    bass2jax.install_neuronx_cc_hook()
    partition_name = nc.partition_id_tensor.name if nc.partition_id_tensor else None
    in_names, out_names, out_avals = [], [], []
    for alloc in nc.m.functions[0].allocations:
        if not isinstance(alloc, mybir.MemoryLocationSet):
            continue
        name = alloc.memorylocations[0].name
        if alloc.kind == "ExternalInput":
            if name != partition_name:
                in_names.append(name)
        elif alloc.kind == "ExternalOutput":
            out_names.append(name)
            out_avals.append(
                jax.core.ShapedArray(tuple(alloc.tensor_shape), mybir.dt.np(alloc.dtype))
            )
    all_in_names = list(in_names) + list(out_names)
    if partition_name is not None:
        all_in_names.append(partition_name)

    def _body(*args):
        operands = list(args)
        if partition_name is not None:
            operands.append(bass2jax.partition_id_tensor())
        return tuple(bass2jax._bass_exec_p.bind(
            *operands,
            out_avals=tuple(out_avals),
            in_names=tuple(all_in_names),
            out_names=tuple(out_names),
            lowering_input_output_aliases=(),
            sim_require_finite=True,
            sim_require_nnan=True,
            nc=nc,
        ))

    devices = jax.devices()[:NCORES]
    mesh = Mesh(np.asarray(devices), ("core",))
    nspec = len(in_names) + len(out_names)
    fn = jax.jit(
        shard_map(
            _body, mesh=mesh,
            in_specs=(PartitionSpec("core"),) * nspec,
            out_specs=(PartitionSpec("core"),) * len(out_names),
            check_rep=False,
        ),
        keep_unused=True,
    )
    # The kernel writes every output element, so the zero "output operand"
    # buffers are never observed; keep them device-resident across calls.
    from jax.sharding import NamedSharding
    sh = NamedSharding(mesh, PartitionSpec("core"))
    dev_zeros = [
        jax.device_put(
            np.zeros((NCORES * a.shape[0], *a.shape[1:]), a.dtype), sh
        )
        for a in out_avals
    ]
    _cache[key] = (fn, in_names, out_names, out_avals, dev_zeros)
    return _cache[key]


def _prep_inputs(x, W, mode):
    # x: [T, B, NI] f32 -> per-core transposed [NI, T, BS]
    xt = np.ascontiguousarray(
        x.reshape(T, NCORES, BS, NI).transpose(1, 3, 0, 2)
    )  # [cores, NI, T, BS]
    wT = np.ascontiguousarray(W.T)  # [NI, NO]
    if mode in ("f32r", "f32"):
        return [{"wT": wT, "xT": xt[c]} for c in range(NCORES)]
    if mode == "f32rx3":
        def rtn11(v):
            b = v.view(np.uint32).astype(np.uint64)
            r = ((b + 0x7FF + ((b >> 12) & 1)) & ~np.uint64(0xFFF)).astype(np.uint32)
            return r.view(np.float32)
        wl = wT - rtn11(wT)
        xl = xt - rtn11(xt)
        return [{"wT": wT, "wTl": wl, "xT": xt[c], "xTl": xl[c]}
                for c in range(NCORES)]
    bf16 = ml_dtypes.bfloat16
    wh = wT.astype(bf16)
    wl = (wT - wh.astype(np.float32)).astype(bf16)
    xh = xt.astype(bf16)
    xl = (xt - xh.astype(np.float32)).astype(bf16)
    return [
        {"wTh": wh, "wTl": wl, "xTh": xh[c], "xTl": xl[c]} for c in range(NCORES)
    ]


def kernel(x, W):
    x = np.asarray(x, dtype=np.float32)
    W = np.asarray(W, dtype=np.float32)
    in_maps = _prep_inputs(x, W, MODE)
    fn, in_names, out_names, out_avals, dev_zeros = _get_runner(MODE)
    args = [
        np.concatenate([np.asarray(in_maps[c][n]) for c in range(NCORES)], axis=0)
        for n in in_names
    ]
    args += dev_zeros
    outs = fn(*args)
    res = {
        n: np.asarray(o).reshape(NCORES, *out_avals[i].shape)
        for i, (n, o) in enumerate(zip(out_names, outs))
    }
    spk = np.empty((T, B, NO), np.float32)
    mem = np.empty((T, B, NO), np.float32)
    for c in range(NCORES):
        # [128, T, OC, BS] -> [T, BS, OC*128]
        spk[:, BS * c:BS * (c + 1), :] = (
            res["spk"][c].transpose(1, 3, 2, 0).reshape(T, BS, NO)
        )
        mem[:, BS * c:BS * (c + 1), :] = (
            res["mem"][c].transpose(1, 3, 2, 0).reshape(T, BS, NO)
        )
    return spk, mem


# revision 15
# speedup vs baseline: 1.1577x; 1.1577x over previous
"""Trainium2 Bass kernel for SNN Linear(2048->1024) + snntorch Leaky (LIF)
layer over T=100 timesteps.

  cur = einsum('tbi,oi->tbo', x, W)
  mem_t = beta*mem_{t-1} + cur_t - heaviside(mem_{t-1} - 1)
  spk_t = heaviside(mem_t - 1)
  returns (spk_rec, mem_rec), both [T, B, NO] float32

Strategy (hardcoded for x:[100,128,2048] f32, W:[1024,2048] f32, 8 cores):
  - Data-parallel over batch: core c handles b in [16c, 16c+16). W replicated.
  - Host pre-transposes x to [core, NI, T, 16] and W to [NI, NO] so both
    matmul operands arrive with the contraction dim on partitions.
  - Matmul runs on TensorE with the weight tile stationary; out tile is
    [o_chunk(128), (t,b)] in PSUM, N=t_block*16 per tile (>=256 keeps
    float32r at full rate).
  - MODE "f32r": single-pass float32r matmuls (TRN2 rounds inputs to 11
    mantissa bits, RTN). MODE "bf16x3": hi/lo bf16 split of both operands,
    3 accumulating passes (drops only the lo*lo term) for ~17-bit mantissa.
  - ScalarE copies PSUM->SBUF into a [128, t, oc, b] block so each timestep
    slice is a contiguous [128,128] tile; VectorE runs the serial LIF
    recurrence as 3 elementwise ops per timestep (same fp32 op order as the
    reference); results stream out per block on the ACT DMA ring.
  - Host re-transposes outputs to [T, B, NO].
"""

import numpy as np
import ml_dtypes

import concourse.bass as bass
import concourse.mybir as mybir
import concourse.tile as tile
from concourse import bacc, bass2jax

# Problem constants (hardcoded per spec)
T, B, NI, NO = 100, 128, 2048, 1024
NCORES = 8
BS = B // NCORES          # 16 batch rows per core
KC = NI // 128            # 16 contraction chunks
OC = NO // 128            # 8 output chunks
BLOCKS = [20, 20, 20, 20, 20]   # timesteps per pipeline block (sum=100)
BETA = 0.9
THRESHOLD = 1.0

MODE = "f32rx3"           # "f32r" (1-pass) or "bf16x3" (hi/lo split, 3-pass)

_cache = {}


def _build_nc(mode, reps=1):
    """reps>1 wraps the whole pipeline in a runtime loop (timing only)."""
    assert sum(BLOCKS) == T and all(b * BS >= 256 for b in BLOCKS)
    f32 = mybir.dt.float32
    nc = bacc.Bacc(None, target_bir_lowering=False)

    if mode in ("f32r", "f32"):
        mmdt = mybir.dt.float32r if mode == "f32r" else mybir.dt.float32
        d_w = [nc.dram_tensor("wT", (NI, NO), mmdt, kind="ExternalInput")]
        d_x = [nc.dram_tensor("xT", (NI, T, BS), mmdt, kind="ExternalInput")]
    else:
        mmdt = mybir.dt.bfloat16
        d_w = [nc.dram_tensor("wTh", (NI, NO), mmdt, kind="ExternalInput"),
               nc.dram_tensor("wTl", (NI, NO), mmdt, kind="ExternalInput")]
        d_x = [nc.dram_tensor("xTh", (NI, T, BS), mmdt, kind="ExternalInput"),
               nc.dram_tensor("xTl", (NI, T, BS), mmdt, kind="ExternalInput")]
    # output layout [p, t, oc, b]: contiguous per-partition streams from the
    # [128, t, (oc b)] SBUF blocks
    d_spk = nc.dram_tensor("spk", (128, T, OC, BS), f32, kind="ExternalOutput")
    d_mem = nc.dram_tensor("mem", (128, T, OC, BS), f32, kind="ExternalOutput")

    # matmul passes: list of (w_tensor_idx, x_tensor_idx)
    passes = [(0, 0)] if mode in ("f32r", "f32") else [(0, 0), (0, 1), (1, 0)]
    tmax = max(BLOCKS)

    with tile.TileContext(nc) as tc:
        with (
            tc.tile_pool(name="w", bufs=1) as wpool,
            tc.tile_pool(name="x", bufs=2) as xpool,
            tc.tile_pool(name="cur", bufs=2) as cpool,
            tc.tile_pool(name="ospk", bufs=2) as spool,
            tc.tile_pool(name="omem", bufs=2) as mpool,
            tc.tile_pool(name="state", bufs=1) as tpool,
            tc.tile_pool(name="psum", bufs=8, space="PSUM") as ppool,
        ):
            w_sb = []
            for wi in range(len(d_w)):
                wt = wpool.tile([128, KC, NO], mmdt, tag=f"w{wi}", name=f"w{wi}")
                w_sb.append(wt)

            def load_w():
                for wi, dw in enumerate(d_w):
                    dvw = dw.ap().rearrange("(c p) m -> p c m", p=128)
                    for c in range(KC):
                        nc.sync.dma_start(out=w_sb[wi][:, c, :], in_=dvw[:, c, :])

            zeros = tpool.tile([128, OC * BS], f32, tag="zeros")
            nc.vector.memset(zeros[:], 0.0)
            m1 = tpool.tile([128, OC * BS], f32, tag="m1")

            def pipeline():
                pm = zeros[:]   # mem_{t-1} slice (starts as zeros)
                ps = zeros[:]   # spk_{t-1} slice
                t0 = 0
                for tblk in BLOCKS:
                    nb = tblk * BS
                    x_sb = []
                    for xi, dx in enumerate(d_x):
                        xt = xpool.tile([128, KC, tmax, BS], mmdt, tag=f"x{xi}")
                        x_sb.append(xt)
                        nc.sync.dma_start(
                            out=xt[:, :, :tblk, :],
                            in_=dx.ap().rearrange("(c p) t b -> p c t b", p=128)[:, :, t0:t0 + tblk, :],
                        )
                    # cur block: oc-major so ACT copies are contiguous
                    cur_sb = cpool.tile([128, OC, tmax, BS], f32, tag="cur")
                    for oc in range(OC):
                        acc = ppool.tile([128, tmax * BS], f32, tag="acc")
                        n_mm = len(passes) * KC
                        i_mm = 0
                        for (wi, xi) in passes:
                            for c in range(KC):
                                nc.tensor.matmul(
                                    acc[:, :nb],
                                    w_sb[wi][:, c, oc * 128:(oc + 1) * 128],
                                    x_sb[xi][:, c, :tblk, :].rearrange("p t b -> p (t b)"),
                                    start=(i_mm == 0), stop=(i_mm == n_mm - 1),
                                )
                                i_mm += 1
                        nc.scalar.copy(
                            out=cur_sb[:, oc, :tblk, :].rearrange("p t b -> p (t b)"),
                            in_=acc[:, :nb],
                        )

                    spk_buf = spool.tile([128, tmax, OC * BS], f32, tag="spk")
                    mem_buf = mpool.tile([128, tmax, OC * BS], f32, tag="mem")
                    for tt in range(tblk):
                        cur_t = cur_sb[:, :, tt, :]   # [128, OC, BS] 3D AP
                        mem_t = mem_buf[:, tt]
                        spk_t = spk_buf[:, tt]
                        # m1 = beta * mem_{t-1} + cur_t
                        nc.vector.scalar_tensor_tensor(
                            out=m1[:], in0=pm, scalar=BETA, in1=cur_t,
                            op0=mybir.AluOpType.mult, op1=mybir.AluOpType.add,
                        )
                        # mem_t = m1 - spk_{t-1}   (reset-by-subtract, thr=1)
                        nc.vector.tensor_tensor(
                            out=mem_t, in0=m1[:], in1=ps, op=mybir.AluOpType.subtract
                        )
                        # spk_t = mem_t > 1.0
                        nc.vector.tensor_scalar(
                            out=spk_t, in0=mem_t, scalar1=THRESHOLD, scalar2=None,
                            op0=mybir.AluOpType.is_gt,
                        )
                        pm = mem_t
                        ps = spk_t

                    # stream block out on the ACT HWDGE ring (separate from
                    # the input ring on SP); fully contiguous per partition
                    nc.scalar.dma_start(
                        out=d_spk.ap()[:, t0:t0 + tblk].rearrange("p t o b -> p t (o b)"),
                        in_=spk_buf[:, :tblk],
                    )
                    nc.scalar.dma_start(
                        out=d_mem.ap()[:, t0:t0 + tblk].rearrange("p t o b -> p t (o b)"),
                        in_=mem_buf[:, :tblk],
                    )
                    t0 += tblk

            if reps == 1:
                load_w()
                pipeline()
            else:
                with tc.For_i(0, reps, 1):
                    load_w()
                    pipeline()

    nc.compile()
    return nc


def _build_nc_f32rx3(reps=1):
    """f32r hi/lo split, 3 accumulating passes, o-halved so both W forms fit
    in SBUF.

    TRN2 rounds float32r values to 11 mantissa bits (RTN) at every f32r
    memory write/read, so DMA-ing the raw fp32 bits into an f32r tile yields
    the hi part for free; the host ships the lo residual (x - RTN11(x)) as a
    second f32r stream.  cur = W_hi.x_hi + W_hi.x_lo + W_lo.x_hi drops only
    the ~2^-24 lo.lo term: fp32-class precision at 3 cyc/row instead of
    native fp32's 4, and full moving-rate at N>=256.
    """
    f32 = mybir.dt.float32
    f32r = mybir.dt.float32r
    nc = bacc.Bacc(None, target_bir_lowering=False)

    d_w = nc.dram_tensor("wT", (NI, NO), f32r, kind="ExternalInput")
    d_wl = nc.dram_tensor("wTl", (NI, NO), f32r, kind="ExternalInput")
    d_x = nc.dram_tensor("xT", (NI, T, BS), f32r, kind="ExternalInput")
    d_xl = nc.dram_tensor("xTl", (NI, T, BS), f32r, kind="ExternalInput")
    d_spk = nc.dram_tensor("spk", (128, T, OC, BS), f32, kind="ExternalOutput")
    d_mem = nc.dram_tensor("mem", (128, T, OC, BS), f32, kind="ExternalOutput")

    OH = OC // 2           # 4 o-chunks per half
    NOH = NO // 2          # 512
    tmax = max(BLOCKS)

    with tile.TileContext(nc) as tc:
        with (
            tc.tile_pool(name="whi", bufs=2) as wpool_hi,
            tc.tile_pool(name="wlo", bufs=1) as wpool_lo,
            tc.tile_pool(name="x", bufs=2) as xpool,
            tc.tile_pool(name="cur", bufs=1) as cpool,
            tc.tile_pool(name="ospk", bufs=2) as spool,
            tc.tile_pool(name="omem", bufs=2) as mpool,
            tc.tile_pool(name="state", bufs=1) as tpool,
            tc.tile_pool(name="psum", bufs=8, space="PSUM") as ppool,
        ):
            zeros = tpool.tile([128, OH * BS], f32, tag="zeros")
            nc.vector.memset(zeros[:], 0.0)
            m1 = tpool.tile([128, OH * BS], f32, tag="m1")

            def sweep(half):
                o0 = half * NOH
                # whi double-buffered: sweep 1's hi-half prefetches during
                # sweep 0. wlo streams at the boundary; kc-outer consumption
                # tolerates its arrival rate.
                w_hi = wpool_hi.tile([128, KC, NOH], f32r, tag="whi", name="whi")
                w_lo = wpool_lo.tile([128, KC, NOH], f32r, tag="wlo", name="wlo")
                dvw = d_w.ap().rearrange("(c p) m -> p c m", p=128)
                dvwl = d_wl.ap().rearrange("(c p) m -> p c m", p=128)
                for c in range(KC):
                    nc.sync.dma_start(out=w_hi[:, c, :], in_=dvw[:, c, o0:o0 + NOH])
                    nc.sync.dma_start(out=w_lo[:, c, :], in_=dvwl[:, c, o0:o0 + NOH])

                pm = zeros[:]
                ps = zeros[:]
                t0 = 0
                for tblk in BLOCKS:
                    nb = tblk * BS
                    x_hi = xpool.tile([128, KC, tmax, BS], f32r, tag="xhi")
                    x_lo = xpool.tile([128, KC, tmax, BS], f32r, tag="xlo")
                    # split per 4-kc group so first matmuls start after ~1/4
                    # of the block's bytes instead of all of them
                    for cg in range(0, KC, 4):
                        for (xt, dx) in ((x_hi, d_x), (x_lo, d_xl)):
                            nc.sync.dma_start(
                                out=xt[:, cg:cg + 4, :tblk, :],
                                in_=dx.ap().rearrange("(c p) t b -> p c t b", p=128)[:, cg:cg + 4, t0:t0 + tblk, :],
                            )

                    cur_sb = cpool.tile([128, OH, tmax, BS], f32, tag="cur")
                    accs = [ppool.tile([128, tmax * BS], f32, tag="acc",
                                       name=f"acc{oc}") for oc in range(OH)]
                    # kc-outer so W/x chunks are consumed at their DMA
                    # arrival rate (removes sweep-start gating)
                    for c in range(KC):
                        for oc in range(OH):
                            for pi, (wt, xt) in enumerate(
                                ((w_hi, x_hi), (w_hi, x_lo), (w_lo, x_hi))
                            ):
                                nc.tensor.matmul(
                                    accs[oc][:, :nb],
                                    wt[:, c, oc * 128:(oc + 1) * 128],
                                    xt[:, c, :tblk, :].rearrange("p t b -> p (t b)"),
                                    start=(c == 0 and pi == 0),
                                    stop=(c == KC - 1 and pi == 2),
                                )
                    for oc in range(OH):
                        nc.scalar.copy(
                            out=cur_sb[:, oc, :tblk, :].rearrange("p t b -> p (t b)"),
                            in_=accs[oc][:, :nb],
                        )

                    spk_buf = spool.tile([128, tmax, OH * BS], f32, tag="spk")
                    mem_buf = mpool.tile([128, tmax, OH * BS], f32, tag="mem")
                    for tt in range(tblk):
                        cur_t = cur_sb[:, :, tt, :]   # [128, OH, BS] 3D AP
                        mem_t = mem_buf[:, tt]
                        spk_t = spk_buf[:, tt]
                        nc.vector.scalar_tensor_tensor(
                            out=m1[:], in0=pm, scalar=BETA, in1=cur_t,
                            op0=mybir.AluOpType.mult, op1=mybir.AluOpType.add,
                        )
                        nc.vector.tensor_tensor(
                            out=mem_t, in0=m1[:], in1=ps, op=mybir.AluOpType.subtract
                        )
                        nc.vector.tensor_scalar(
                            out=spk_t, in0=mem_t, scalar1=THRESHOLD, scalar2=None,
                            op0=mybir.AluOpType.is_gt,
                        )
                        pm = mem_t
                        ps = spk_t

                    th = tblk // 2
                    for (lo, hi) in ((0, th), (th, tblk)):
                        nc.scalar.dma_start(
                            out=d_spk.ap()[:, t0 + lo:t0 + hi, 4 * half:4 * half + OH, :]
                                .rearrange("p t o b -> p t (o b)"),
                            in_=spk_buf[:, lo:hi],
                        )
                        nc.scalar.dma_start(
                            out=d_mem.ap()[:, t0 + lo:t0 + hi, 4 * half:4 * half + OH, :]
                                .rearrange("p t o b -> p t (o b)"),
                            in_=mem_buf[:, lo:hi],
                        )
                    t0 += tblk

            def whole():
                sweep(0)
                sweep(1)

            if reps == 1:
                whole()
            else:
                with tc.For_i(0, reps, 1):
                    whole()

    nc.compile()
    return nc


def _get_nc(mode):
    if mode not in _cache:
        _cache[mode] = (_build_nc_f32rx3() if mode == "f32rx3" else _build_nc(mode))
    return _cache[mode]


def _get_runner(mode):
    """Jitted SPMD callable for the mode's Bass program, cached across calls.

    Mirrors concourse.bass_utils.run_bass_kernel_spmd's axon path but keeps
    the jax.jit object alive so repeated kernel() calls skip retracing.
    """
    key = ("runner", mode)
    if key in _cache:
        return _cache[key]
    import jax
    from jax.experimental.shard_map import shard_map
    from jax.sharding import Mesh, PartitionSpec

    nc = _get_nc(mode)
    bass2jax.install_neuronx_cc_hook()
    partition_name = nc.partition_id_tensor.name if nc.partition_id_tensor else None
    in_names, out_names, out_avals = [], [], []
    for alloc in nc.m.functions[0].allocations:
        if not isinstance(alloc, mybir.MemoryLocationSet):
            continue
        name = alloc.memorylocations[0].name
        if alloc.kind == "ExternalInput":
            if name != partition_name:
                in_names.append(name)
        elif alloc.kind == "ExternalOutput":
            out_names.append(name)
            out_avals.append(
                jax.core.ShapedArray(tuple(alloc.tensor_shape), mybir.dt.np(alloc.dtype))
            )
    all_in_names = list(in_names) + list(out_names)
    if partition_name is not None:
        all_in_names.append(partition_name)

    def _body(*args):
        operands = list(args)
        if partition_name is not None:
            operands.append(bass2jax.partition_id_tensor())
        return tuple(bass2jax._bass_exec_p.bind(
            *operands,
            out_avals=tuple(out_avals),
            in_names=tuple(all_in_names),
            out_names=tuple(out_names),
            lowering_input_output_aliases=(),
            sim_require_finite=True,
            sim_require_nnan=True,
            nc=nc,
        ))

    devices = jax.devices()[:NCORES]
    mesh = Mesh(np.asarray(devices), ("core",))
    nspec = len(in_names) + len(out_names)
    fn = jax.jit(
        shard_map(
            _body, mesh=mesh,
            in_specs=(PartitionSpec("core"),) * nspec,
            out_specs=(PartitionSpec("core"),) * len(out_names),
            check_rep=False,
        ),
        keep_unused=True,
    )
    # The kernel writes every output element, so the zero "output operand"
    # buffers are never observed; keep them device-resident across calls.
    from jax.sharding import NamedSharding
    sh = NamedSharding(mesh, PartitionSpec("core"))
    dev_zeros = [
        jax.device_put(
            np.zeros((NCORES * a.shape[0], *a.shape[1:]), a.dtype), sh
        )
        for a in out_avals
    ]
    _cache[key] = (fn, in_names, out_names, out_avals, dev_zeros)
    return _cache[key]


def _prep_inputs(x, W, mode):
    # x: [T, B, NI] f32 -> per-core transposed [NI, T, BS]
    xt = np.ascontiguousarray(
        x.reshape(T, NCORES, BS, NI).transpose(1, 3, 0, 2)
    )  # [cores, NI, T, BS]
    wT = np.ascontiguousarray(W.T)  # [NI, NO]
    if mode in ("f32r", "f32"):
        return [{"wT": wT, "xT": xt[c]} for c in range(NCORES)]
    if mode == "f32rx3":
        def rtn11(v):
            b = v.view(np.uint32).astype(np.uint64)
            r = ((b + 0x7FF + ((b >> 12) & 1)) & ~np.uint64(0xFFF)).astype(np.uint32)
            return r.view(np.float32)
        wl = wT - rtn11(wT)
        xl = xt - rtn11(xt)
        return [{"wT": wT, "wTl": wl, "xT": xt[c], "xTl": xl[c]}
                for c in range(NCORES)]
    bf16 = ml_dtypes.bfloat16
    wh = wT.astype(bf16)
    wl = (wT - wh.astype(np.float32)).astype(bf16)
    xh = xt.astype(bf16)
    xl = (xt - xh.astype(np.float32)).astype(bf16)
    return [
        {"wTh": wh, "wTl": wl, "xTh": xh[c], "xTl": xl[c]} for c in range(NCORES)
    ]


def kernel(x, W):
    x = np.asarray(x, dtype=np.float32)
    W = np.asarray(W, dtype=np.float32)
    in_maps = _prep_inputs(x, W, MODE)
    fn, in_names, out_names, out_avals, dev_zeros = _get_runner(MODE)
    args = [
        np.concatenate([np.asarray(in_maps[c][n]) for c in range(NCORES)], axis=0)
        for n in in_names
    ]
    args += dev_zeros
    try:
        outs = fn(*args)
        outs = [np.asarray(o) for o in outs]
    except Exception:
        # the axon terminal occasionally reports a transient
        # NRT_EXEC_UNIT_UNRECOVERABLE on the first execution after another
        # process's teardown; give it a moment and retry once
        import time as _time
        _time.sleep(25)
        outs = fn(*args)
        outs = [np.asarray(o) for o in outs]
    res = {
        n: o.reshape(NCORES, *out_avals[i].shape)
        for i, (n, o) in enumerate(zip(out_names, outs))
    }
    spk = np.empty((T, B, NO), np.float32)
    mem = np.empty((T, B, NO), np.float32)
    for c in range(NCORES):
        # [128, T, OC, BS] -> [T, BS, OC*128]
        spk[:, BS * c:BS * (c + 1), :] = (
            res["spk"][c].transpose(1, 3, 2, 0).reshape(T, BS, NO)
        )
        mem[:, BS * c:BS * (c + 1), :] = (
            res["mem"][c].transpose(1, 3, 2, 0).reshape(T, BS, NO)
        )
    return spk, mem
